# revision 1
# baseline (speedup 1.0000x reference)
"""HGAT-ESM2 Bass kernel for 8 trn2 NeuronCores.

Key mathematical simplification: the reference's TypeAttention computes
seg_softmax(logit[d], d) where logit is per-dst-node -- values are constant
within each segment, so alpha_e = 1/indegree_r(dst_e), host-computable from
the indices alone.  The mul_W/mur_W/attn_w machinery cancels out entirely.

Remaining math per layer l (x = hom node features, table [N_hom, H]):
  hl = x @ Wl.T ; hr = x @ Wr.T
  s_e = sum_j leaky(hl[src_e, j] + hr[dst_e, j]) ; e_e = alpha_e * s_e
  a = seg_softmax(e, dst, N_hom) ; xo = segment_sum(a * x[src], dst)

Sharding: 61440 padded hom nodes = 8 shards x 7680 rows (5120 p + 2560 g per
core, interleaved renumbering).  Per layer one AllGather of the [x || hl]
table (61440 x 1024 f32); edge work is dst-block-sharded with uniform
per-core program structure (40 p-blocks x 7 chunks + 20 g-blocks x 8 chunks
in L1; 40 p-blocks x 7 chunks in L2 -- only pp/gp relations matter in the
last layer since only protein outputs survive).  Segment softmax + scatter
via one-hot matmuls on the PE; leaky+rowsum fused on ACT (Prelu+accum_out).
"""
import sys
sys.path.insert(0, "/opt/trn_rl_repo")
import numpy as np
import concourse.bass as bass
import concourse.bacc as bacc
import concourse.mybir as mybir
import concourse.tile as tile
from concourse.bass_utils import run_bass_kernel_spmd
from concourse.masks import make_identity

F32 = mybir.dt.float32
I32 = mybir.dt.int32
P = 128
NCORES = 8
NpR, NgR = 40000, 20000          # real node counts
PSH, GSH = 5120, 2560            # per-core padded p/g slots
SH = PSH + GSH                   # 7680 rows per shard
NH = SH * NCORES                 # 61440 padded hom nodes
PPC, GPC = NpR // NCORES, NgR // NCORES   # 5000 / 2500 real rows per core
NB_P, NB_G = PSH // P, GSH // P  # 40 / 20 blocks per core
C_P, C_G = 7, 8                  # chunk capacity per p-/g-block
T1 = NB_P * C_P + NB_G * C_G     # 440 chunk slots per core, layer 1
T2 = NB_P * C_P                  # 280 chunk slots per core, layer 2
PAD_ROW = NH - 1                 # a guaranteed-zero row in the xh tables
H, Dm, De, CLS = 512, 256, 1280, 1024
EPS = 1e-5
SLOPE = 0.2
AX = mybir.AxisListType.X
OP = mybir.AluOpType
AF = mybir.ActivationFunctionType


def _hom_ids(ids, is_g):
    ids = np.asarray(ids, np.int64)
    if is_g:
        c, r = ids // GPC, ids % GPC
        return c * SH + PSH + r
    c, r = ids // PPC, ids % PPC
    return c * SH + r


def _build_meta(rel_list, nb_g):
    """rel_list: list of (src_hom, dst_hom, w_alpha). Returns per-core dicts."""
    src = np.concatenate([r[0] for r in rel_list])
    dst = np.concatenate([r[1] for r in rel_list])
    w = np.concatenate([r[2] for r in rel_list]).astype(np.float32)
    core = dst // SH
    loc = dst % SH
    blk = loc // P
    dloc = (loc % P).astype(np.float32)
    nblk = NB_P + nb_g
    ncols = NB_P * C_P + nb_g * C_G
    metas = []
    for c in range(NCORES):
        sel = np.where(core == c)[0]
        g_idx = np.full((P, ncols), PAD_ROW, np.int32)
        d_loc = np.full((P, ncols), -1.0, np.float32)
        w_a = np.zeros((P, ncols), np.float32)
        order = sel[np.argsort(blk[sel], kind="stable")]
        bvals = blk[order]
        bounds = np.searchsorted(bvals, np.arange(nblk + 1))
        for b in range(nblk):
            e = order[bounds[b]:bounds[b + 1]]
            cap = C_P if b < NB_P else C_G
            col0 = b * C_P if b < NB_P else NB_P * C_P + (b - NB_P) * C_G
            k = len(e)
            assert k <= cap * P, f"block overflow: {k} > {cap * P}"
            lanes, chunks = np.arange(k) % P, np.arange(k) // P
            g_idx[lanes, col0 + chunks] = src[e]
            d_loc[lanes, col0 + chunks] = dloc[e]
            w_a[lanes, col0 + chunks] = w[e]
        # transposed one-hot per chunk: ohme[m, j*128 + e] = (d_loc[e, j] == m)
        ohme = (d_loc[None, :, :] == np.arange(P, dtype=np.float32)[:, None, None])
        ohme = ohme.astype(np.float32).transpose(0, 2, 1).reshape(P, ncols * P)
        metas.append(dict(g_idx=g_idx, d_loc=d_loc, w_a=w_a,
                          ohme=np.ascontiguousarray(ohme)))
    return metas


def _pad_rows(a, n):
    out = np.zeros((n, a.shape[1]), a.dtype)
    out[: a.shape[0]] = np.asarray(a, np.float32)
    return out


def _kn(W):
    """[K, N] -> [128, (K//128)*N] with chunk k of rows at columns [k*N:(k+1)*N]"""
    K, N = W.shape
    return np.ascontiguousarray(
        W.reshape(K // P, P, N).transpose(1, 0, 2).reshape(P, -1).astype(np.float32))


def _bc(row):
    return np.ascontiguousarray(
        np.tile(np.asarray(row, np.float32)[None, :], (P, 1)))


def build_program():
    nc = bacc.Bacc("TRN2", target_bir_lowering=False, debug=False,
                   num_devices=NCORES)

    def inp(name, shape, dt=F32):
        return nc.dram_tensor(name, list(shape), dt, kind="ExternalInput").ap()

    xpm = inp("xpm", [PSH, Dm]); xpe = inp("xpe", [PSH, De])
    xgm = inp("xgm", [GSH, Dm]); xge = inp("xge", [GSH, De])
    wnames = ["wm_p", "wm_g", "we_p", "we_g",
              "bfuse_p", "bfuse_g", "gbc_p", "gbc_g", "bbc_p", "bbc_g",
              "wl1_p", "wl1_g", "wr1_p", "wr1_g",
              "bl1_p", "bl1_g", "br1_p", "br1_g"]
    wshapes = {"wm_p": [P, (Dm // P) * (H // 2)], "wm_g": [P, (Dm // P) * (H // 2)],
               "we_p": [P, (De // P) * (H // 2)], "we_g": [P, (De // P) * (H // 2)],
               "bfuse_p": [P, H], "bfuse_g": [P, H],
               "gbc_p": [P, H], "gbc_g": [P, H],
               "bbc_p": [P, H], "bbc_g": [P, H],
               "wl1_p": [P, (H // P) * H], "wl1_g": [P, (H // P) * H],
               "wr1_p": [P, (H // P) * H], "wr1_g": [P, (H // P) * H],
               "bl1_p": [P, H], "bl1_g": [P, H],
               "br1_p": [P, H], "br1_g": [P, H]}
    wap = {n: inp(n, wshapes[n]) for n in wnames}
    wl2 = inp("wl2", [P, (H // P) * H]); wr2 = inp("wr2", [P, (H // P) * H])
    wcls = inp("wcls", [P, (H // P) * CLS]); bcls_bc = inp("bcls_bc", [P, CLS])
    iota_in = inp("iota_in", [P, P])
    gi1 = inp("gi1", [P, T1], I32); dl1 = inp("dl1", [P, T1]); wa1 = inp("wa1", [P, T1])
    om1 = inp("om1", [P, T1 * P])
    gi2 = inp("gi2", [P, T2], I32); dl2 = inp("dl2", [P, T2]); wa2 = inp("wa2", [P, T2])
    om2 = inp("om2", [P, T2 * P])
    out_d = nc.dram_tensor("out", [PSH, CLS], F32, kind="ExternalOutput").ap()

    with tile.TileContext(nc, num_cores=NCORES) as tc:
        with (
            tc.tile_pool(name="dram", bufs=1, space="DRAM") as dram,
            tc.tile_pool(name="const", bufs=1) as cpool,
        ):
            xh1_sh = dram.tile([SH, 2 * H], F32)
            xh1_full = dram.tile([NH, 2 * H], F32, addr_space="Shared")
            xh2_sh = dram.tile([SH, 2 * H], F32)
            xh2_full = dram.tile([NH, 2 * H], F32, addr_space="Shared")
            hr1_tab = dram.tile([SH, H], F32)

            iota = cpool.tile([P, P], F32)
            nc.sync.dma_start(out=iota[:], in_=iota_in[:])
            ident = cpool.tile([P, P], F32)
            make_identity(nc, ident[:])

            meta_t = {}
            for name, ap, dt in (("gi1", gi1, I32), ("dl1", dl1, F32),
                                 ("wa1", wa1, F32), ("gi2", gi2, I32),
                                 ("dl2", dl2, F32), ("wa2", wa2, F32)):
                t = cpool.tile(list(ap.shape), dt, name=f"m_{name}")
                nc.sync.dma_start(out=t[:], in_=ap[:])
                meta_t[name] = t

            def transposes(src_sb, psum_pool, sbuf_pool, nchunks, tag):
                """PE-transpose [128, n*128] -> list of [128,128] sbuf tiles."""
                outs = []
                for k in range(nchunks):
                    ps = psum_pool.tile([P, P], F32, name=f"tp_{tag}",
                                        tag="small", space="PSUM")
                    nc.tensor.transpose(out=ps[:], in_=src_sb[:, k * P:(k + 1) * P],
                                        identity=ident[:])
                    sb = sbuf_pool.tile([P, P], F32, name=f"ts_{tag}{k}",
                                        tag=f"ts_{tag}{k}")
                    nc.vector.tensor_copy(out=sb[:], in_=ps[:])
                    outs.append(sb)
                return outs

            def layernorm(x_sb, pool, tag):
                """returns xn tile = (x - mean)/sqrt(var+eps), per-row stats."""
                s1 = pool.tile([P, 1], F32, name=f"{tag}s1", tag=f"{tag}s1")
                nc.vector.reduce_sum(out=s1[:], in_=x_sb[:], axis=AX)
                sq = pool.tile([P, H], F32, name=f"{tag}sq", tag=f"{tag}sq")
                ssq = pool.tile([P, 1], F32, name=f"{tag}ssq", tag=f"{tag}ssq")
                nc.scalar.activation(out=sq[:], in_=x_sb[:], func=AF.Square,
                                     accum_out=ssq[:])
                mu = pool.tile([P, 1], F32, name=f"{tag}mu", tag=f"{tag}mu")
                nc.vector.tensor_scalar_mul(out=mu[:], in0=s1[:], scalar1=1.0 / H)
                var = pool.tile([P, 1], F32, name=f"{tag}var", tag=f"{tag}var")
                nc.vector.tensor_scalar_mul(out=var[:], in0=ssq[:], scalar1=1.0 / H)
                mu2 = pool.tile([P, 1], F32, name=f"{tag}mu2", tag=f"{tag}mu2")
                nc.vector.tensor_tensor(out=mu2[:], in0=mu[:], in1=mu[:], op=OP.mult)
                nc.vector.tensor_tensor(out=var[:], in0=var[:], in1=mu2[:],
                                        op=OP.subtract)
                nc.vector.tensor_scalar_add(out=var[:], in0=var[:], scalar1=EPS)
                rv = pool.tile([P, 1], F32, name=f"{tag}rv", tag=f"{tag}rv")
                nc.vector.reciprocal(out=rv[:], in_=var[:])
                rstd = pool.tile([P, 1], F32, name=f"{tag}rstd", tag=f"{tag}rstd")
                nc.scalar.activation(out=rstd[:], in_=rv[:], func=AF.Sqrt)
                xn = pool.tile([P, H], F32, name=f"{tag}xn", tag=f"{tag}xn")
                nc.vector.tensor_scalar(out=xn[:], in0=x_sb[:], scalar1=mu[:],
                                        scalar2=rstd[:], op0=OP.subtract,
                                        op1=OP.mult)
                return xn

            # ================= FUSE + layer-1 hl/hr =================
            with (
                tc.tile_pool(name="fw", bufs=1) as fw,
                tc.tile_pool(name="fsb", bufs=2) as fsb,
                tc.tile_pool(name="fps", bufs=2, space="PSUM") as fps,
            ):
                W = {}
                for n in wnames:
                    t = fw.tile(list(wshapes[n]), F32, name=f"c_{n}")
                    nc.sync.dma_start(out=t[:], in_=wap[n][:])
                    W[n] = t
                zero_row = fw.tile([P, 2 * H], F32)
                nc.vector.memset(zero_row[:], 0.0)

                for b in range(NB_P + NB_G):
                    sfx = "p" if b < NB_P else "g"
                    r0 = b * P if b < NB_P else (b - NB_P) * P
                    xm_src, xe_src = (xpm, xpe) if b < NB_P else (xgm, xge)
                    xm = fsb.tile([P, Dm], F32, tag="xm", name="xm")
                    xe = fsb.tile([P, De], F32, tag="xe", name="xe")
                    nc.sync.dma_start(out=xm[:], in_=xm_src[r0:r0 + P, :])
                    nc.sync.dma_start(out=xe[:], in_=xe_src[r0:r0 + P, :])
                    xmT = transposes(xm, fps, fsb, Dm // P, "xm")
                    xeT = transposes(xe, fps, fsb, De // P, "xe")
                    hps = fps.tile([P, H], F32, tag="hps", space="PSUM", name="hps")
                    for k in range(Dm // P):
                        nc.tensor.matmul(out=hps[:, :H // 2], lhsT=xmT[k][:],
                                         rhs=W[f"wm_{sfx}"][:, k * (H // 2):(k + 1) * (H // 2)],
                                         start=(k == 0), stop=(k == Dm // P - 1))
                    for k in range(De // P):
                        nc.tensor.matmul(out=hps[:, H // 2:], lhsT=xeT[k][:],
                                         rhs=W[f"we_{sfx}"][:, k * (H // 2):(k + 1) * (H // 2)],
                                         start=(k == 0), stop=(k == De // P - 1))
                    cat = fsb.tile([P, H], F32, tag="cat", name="cat")
                    nc.vector.tensor_tensor(out=cat[:], in0=hps[:],
                                            in1=W[f"bfuse_{sfx}"][:], op=OP.add)
                    xn = layernorm(cat, fsb, "f_")
                    h1 = fsb.tile([P, H], F32, tag="h1", name="h1")
                    nc.vector.tensor_tensor(out=h1[:], in0=xn[:],
                                            in1=W[f"gbc_{sfx}"][:], op=OP.mult)
                    nc.vector.tensor_tensor(out=h1[:], in0=h1[:],
                                            in1=W[f"bbc_{sfx}"][:], op=OP.add)
                    nc.sync.dma_start(out=xh1_sh[b * P:(b + 1) * P, 0:H], in_=h1[:])
                    xnT = transposes(xn, fps, fsb, H // P, "xn")
                    for wnm, bnm, dst_ap in (
                        ("wl1", "bl1", xh1_sh[b * P:(b + 1) * P, H:2 * H]),
                        ("wr1", "br1", hr1_tab[b * P:(b + 1) * P, :]),
                    ):
                        ps = fps.tile([P, H], F32, tag="hlr", space="PSUM", name="hlr")
                        for k in range(H // P):
                            nc.tensor.matmul(out=ps[:], lhsT=xnT[k][:],
                                             rhs=W[f"{wnm}_{sfx}"][:, k * H:(k + 1) * H],
                                             start=(k == 0), stop=(k == H // P - 1))
                        sb = fsb.tile([P, H], F32, tag="hlr_sb", name="hlr_sb")
                        nc.vector.tensor_tensor(out=sb[:], in0=ps[:],
                                                in1=W[f"{bnm}_{sfx}"][:], op=OP.add)
                        nc.sync.dma_start(out=dst_ap, in_=sb[:])
                # zero the pad rows (so PAD_ROW gathers and pad dst reads are clean)
                for lo, hi in ((PPC, PSH), (PSH + GPC, SH)):
                    r = lo
                    while r < hi:
                        n = min(P, hi - r)
                        nc.sync.dma_start(out=xh1_sh[r:r + n, :], in_=zero_row[:n, :])
                        nc.sync.dma_start(out=xh2_sh[r:r + n, :], in_=zero_row[:n, :])
                        nc.sync.dma_start(out=hr1_tab[r:r + n, :],
                                          in_=zero_row[:n, :H])
                        r += n

            nc.gpsimd.collective_compute(
                "AllGather", OP.bypass, replica_groups=[list(range(NCORES))],
                ins=[xh1_sh.opt()], outs=[xh1_full.opt()])

            # ================= edge phase (both layers) =================
            def edge_phase(layer, xh_full, nblocks, caps, gi, dl, wa, om_ap,
                           hr_fn, epilogue):
                with (
                    tc.tile_pool(name=f"esb{layer}", bufs=2) as esb,
                    tc.tile_pool(name=f"exs{layer}", bufs=2) as exs,
                    tc.tile_pool(name=f"eps{layer}", bufs=2, space="PSUM") as eps,
                    tc.tile_pool(name=f"exo{layer}", bufs=1, space="PSUM") as exo,
                ):
                    col0 = 0
                    for b in range(nblocks):
                        C = caps[b]
                        hr_sb = hr_fn(b, esb, eps)
                        xs = exs.tile([P, C_G * 2 * H], F32, tag="xs", name="xs")
                        for j in range(C):
                            nc.gpsimd.indirect_dma_start(
                                out=xs[:, j * 2 * H:(j + 1) * 2 * H],
                                out_offset=None, in_=xh_full[:],
                                in_offset=bass.IndirectOffsetOnAxis(
                                    ap=meta_t[gi][:, col0 + j:col0 + j + 1], axis=0))
                        om_blk = esb.tile([P, C_G * P], F32, tag="om", name="om_blk")
                        nc.sync.dma_start(out=om_blk[:, :C * P],
                                          in_=om_ap[:, col0 * P:(col0 + C) * P])
                        e_all = esb.tile([P, C_G], F32, tag="e_all", name="e_all")
                        u_all = esb.tile([P, C_G], F32, tag="u_all", name="u_all")
                        oh_all = esb.tile([P, C_G * P], F32, tag="oh_all", name="oh_all")
                        kmax = esb.tile([P, 1], F32, tag="kmax", name="kmax")
                        nc.vector.memset(kmax[:], 0.0)
                        for j in range(C):
                            he = eps.tile([P, H], F32, tag="big", space="PSUM", name="he")
                            nc.tensor.matmul(out=he[:], lhsT=om_blk[:, j * P:(j + 1) * P],
                                             rhs=hr_sb[:], start=True, stop=True)
                            t = esb.tile([P, H], F32, tag="t", name="t")
                            nc.vector.tensor_tensor(
                                out=t[:], in0=xs[:, j * 2 * H + H:(j + 1) * 2 * H],
                                in1=he[:], op=OP.add)
                            lk = esb.tile([P, H], F32, tag="lk", name="lk")
                            s_col = esb.tile([P, 1], F32, tag="s_col", name="s_col")
                            nc.scalar.activation(out=lk[:], in_=t[:], func=AF.Prelu,
                                                 alpha=SLOPE, accum_out=s_col[:])
                            nc.vector.tensor_tensor(
                                out=e_all[:, j:j + 1], in0=s_col[:],
                                in1=meta_t[wa][:, col0 + j:col0 + j + 1], op=OP.mult)
                            nc.vector.tensor_tensor(
                                out=oh_all[:, j * P:(j + 1) * P],
                                in0=meta_t[dl][:, col0 + j:col0 + j + 1].to_broadcast([P, P]),
                                in1=iota[:], op=OP.is_equal)
                            m2 = esb.tile([P, P], F32, tag="m2", name="m2")
                            nc.vector.tensor_scalar(
                                out=m2[:], in0=oh_all[:, j * P:(j + 1) * P],
                                scalar1=e_all[:, j:j + 1], scalar2=None, op0=OP.mult)
                            m2T = eps.tile([P, P], F32, tag="small", space="PSUM",
                                           name="m2T")
                            nc.tensor.transpose(out=m2T[:], in_=m2[:], identity=ident[:])
                            cmax = esb.tile([P, 1], F32, tag="cmax", name="cmax")
                            nc.vector.tensor_reduce(out=cmax[:], in_=m2T[:], axis=AX,
                                                    op=OP.max)
                            nc.vector.tensor_tensor(out=kmax[:], in0=kmax[:],
                                                    in1=cmax[:], op=OP.max)
                        negk = esb.tile([P, 1], F32, tag="negk", name="negk")
                        nc.vector.tensor_scalar_mul(out=negk[:], in0=kmax[:],
                                                    scalar1=-1.0)
                        den = exo.tile([P, 1], F32, tag="den", space="PSUM", name="den")
                        for j in range(C):
                            nke = eps.tile([P, 1], F32, tag="small", space="PSUM",
                                           name="nke")
                            nc.tensor.matmul(out=nke[:], lhsT=om_blk[:, j * P:(j + 1) * P],
                                             rhs=negk[:], start=True, stop=True)
                            nke_sb = esb.tile([P, 1], F32, tag="nke_sb", name="nke_sb")
                            nc.vector.tensor_copy(out=nke_sb[:], in_=nke[:])
                            nc.scalar.activation(out=u_all[:, j:j + 1],
                                                 in_=e_all[:, j:j + 1], func=AF.Exp,
                                                 bias=nke_sb[:])
                            nc.tensor.matmul(out=den[:], lhsT=oh_all[:, j * P:(j + 1) * P],
                                             rhs=u_all[:, j:j + 1],
                                             start=(j == 0), stop=(j == C - 1))
                        den_sb = esb.tile([P, 1], F32, tag="den_sb", name="den_sb")
                        nc.vector.tensor_scalar_max(out=den_sb[:], in0=den[:],
                                                    scalar1=1e-30)
                        rden = esb.tile([P, 1], F32, tag="rden", name="rden")
                        nc.vector.reciprocal(out=rden[:], in_=den_sb[:])
                        xo_ps = exo.tile([P, H], F32, tag="xo", space="PSUM", name="xo")
                        for j in range(C):
                            rde = eps.tile([P, 1], F32, tag="small", space="PSUM",
                                           name="rde")
                            nc.tensor.matmul(out=rde[:], lhsT=om_blk[:, j * P:(j + 1) * P],
                                             rhs=rden[:], start=True, stop=True)
                            a_col = esb.tile([P, 1], F32, tag="a_col", name="a_col")
                            nc.vector.tensor_tensor(out=a_col[:], in0=u_all[:, j:j + 1],
                                                    in1=rde[:], op=OP.mult)
                            oha = esb.tile([P, P], F32, tag="oha", name="oha")
                            nc.vector.tensor_scalar(out=oha[:],
                                                    in0=oh_all[:, j * P:(j + 1) * P],
                                                    scalar1=a_col[:], scalar2=None,
                                                    op0=OP.mult)
                            nc.tensor.matmul(out=xo_ps[:], lhsT=oha[:],
                                             rhs=xs[:, j * 2 * H:j * 2 * H + H],
                                             start=(j == 0), stop=(j == C - 1))
                        xo_sb = esb.tile([P, H], F32, tag="xo_sb", name="xo_sb")
                        nc.vector.tensor_copy(out=xo_sb[:], in_=xo_ps[:])
                        epilogue(b, xo_sb, esb, eps)
                        col0 += C

            with tc.tile_pool(name="l2w", bufs=1) as l2w:
                wl2_t = l2w.tile([P, (H // P) * H], F32, name="wl2_t")
                nc.sync.dma_start(out=wl2_t[:], in_=wl2[:])
                wr2_t = l2w.tile([P, (H // P) * H], F32, name="wr2_t")
                nc.sync.dma_start(out=wr2_t[:], in_=wr2[:])

                # ---- layer 1 ----
                def hr1_fn(b, esb, eps):
                    hr_sb = esb.tile([P, H], F32, tag="hr_sb", name="hr_sb")
                    nc.sync.dma_start(out=hr_sb[:], in_=hr1_tab[b * P:(b + 1) * P, :])
                    return hr_sb

                def epi1(b, xo_sb, esb, eps):
                    nc.sync.dma_start(out=xh2_sh[b * P:(b + 1) * P, 0:H], in_=xo_sb[:])
                    xoT = transposes(xo_sb, eps, esb, H // P, "xoT")
                    ps = eps.tile([P, H], F32, tag="big", space="PSUM", name="hl2ps")
                    for k in range(H // P):
                        nc.tensor.matmul(out=ps[:], lhsT=xoT[k][:],
                                         rhs=wl2_t[:, k * H:(k + 1) * H],
                                         start=(k == 0), stop=(k == H // P - 1))
                    sb = esb.tile([P, H], F32, tag="hl2sb", name="hl2sb")
                    nc.vector.tensor_copy(out=sb[:], in_=ps[:])
                    nc.sync.dma_start(out=xh2_sh[b * P:(b + 1) * P, H:2 * H], in_=sb[:])

                edge_phase(1, xh1_full, NB_P + NB_G, [C_P] * NB_P + [C_G] * NB_G,
                           "gi1", "dl1", "wa1", om1, hr1_fn, epi1)

                nc.gpsimd.collective_compute(
                    "AllGather", OP.bypass, replica_groups=[list(range(NCORES))],
                    ins=[xh2_sh.opt()], outs=[xh2_full.opt()])

                # ---- layer 2 + head LN + cls ----
                with tc.tile_pool(name="clsw", bufs=1) as clsw:
                    wcls_t = clsw.tile([P, (H // P) * CLS], F32, name="wcls_t")
                    nc.sync.dma_start(out=wcls_t[:], in_=wcls[:])
                    bcls_t = clsw.tile([P, CLS], F32, name="bcls_t")
                    nc.sync.dma_start(out=bcls_t[:], in_=bcls_bc[:])

                    def hr2_fn(b, esb, eps):
                        xr = esb.tile([P, H], F32, tag="xr", name="xr")
                        nc.sync.dma_start(out=xr[:],
                                          in_=xh2_sh[b * P:(b + 1) * P, 0:H])
                        xrT = transposes(xr, eps, esb, H // P, "xrT")
                        ps = eps.tile([P, H], F32, tag="big", space="PSUM",
                                      name="hr2ps")
                        for k in range(H // P):
                            nc.tensor.matmul(out=ps[:], lhsT=xrT[k][:],
                                             rhs=wr2_t[:, k * H:(k + 1) * H],
                                             start=(k == 0), stop=(k == H // P - 1))
                        hr_sb = esb.tile([P, H], F32, tag="hr_sb", name="hr_sb")
                        nc.vector.tensor_copy(out=hr_sb[:], in_=ps[:])
                        return hr_sb

                    def epi2(b, xo_sb, esb, eps):
                        xn = layernorm(xo_sb, esb, "c_")
                        xnT = transposes(xn, eps, esb, H // P, "c_xnT")
                        for half in range(2):
                            ps = eps.tile([P, H], F32, tag="big", space="PSUM",
                                          name="clsps")
                            for k in range(H // P):
                                nc.tensor.matmul(
                                    out=ps[:], lhsT=xnT[k][:],
                                    rhs=wcls_t[:, k * CLS + half * H:
                                               k * CLS + (half + 1) * H],
                                    start=(k == 0), stop=(k == H // P - 1))
                            ob = esb.tile([P, H], F32, tag="c_ob", name="c_ob")
                            nc.vector.tensor_tensor(
                                out=ob[:], in0=ps[:],
                                in1=bcls_t[:, half * H:(half + 1) * H], op=OP.add)
                            nc.sync.dma_start(
                                out=out_d[b * P:(b + 1) * P,
                                          half * H:(half + 1) * H],
                                in_=ob[:])

                    edge_phase(2, xh2_full, NB_P, [C_P] * NB_P,
                               "gi2", "dl2", "wa2", om2, hr2_fn, epi2)

    nc.compile()
    return nc


def kernel(_run_kwargs=None, **inputs):
    run_kwargs = _run_kwargs or {}
    inp = {k: np.asarray(v) for k, v in inputs.items()}

    def rel_arrays(st, dt, s, d):
        nt = NpR if dt == 0 else NgR
        ideg = np.bincount(d, minlength=nt)
        w = (1.0 / np.maximum(ideg[d], 1)).astype(np.float32)
        return (_hom_ids(s, st == 1), _hom_ids(d, dt == 1), w)

    rels = [(0, 0, inp["pp_src"], inp["pp_dst"]),
            (0, 1, inp["pg_src"], inp["pg_dst"]),
            (1, 0, inp["gp_src"], inp["gp_dst"]),
            (1, 1, inp["gg_src"], inp["gg_dst"])]
    rel1 = [rel_arrays(*r) for r in rels]
    meta1 = _build_meta(rel1, NB_G)
    rel2 = [rel_arrays(*r) for r in rels if r[1] == 0]
    meta2 = _build_meta(rel2, 0)

    lnf_p_g, lnf_p_b = inp["lnf_p_g"], inp["lnf_p_b"]
    lnf_g_g, lnf_g_b = inp["lnf_g_g"], inp["lnf_g_b"]
    wl1 = np.ascontiguousarray(inp["node_Wl"][0].T, dtype=np.float32)
    wr1 = np.ascontiguousarray(inp["node_Wr"][0].T, dtype=np.float32)
    wl2 = np.ascontiguousarray(inp["node_Wl"][1].T, dtype=np.float32)
    wr2 = np.ascontiguousarray(inp["node_Wr"][1].T, dtype=np.float32)
    hg, hb = inp["head_ln_g"], inp["head_ln_b"]
    wcls_T = np.ascontiguousarray(inp["Wcls"].T, dtype=np.float32)

    common = dict(
        wm_p=_kn(np.ascontiguousarray(inp["Wmsa_p"].T, np.float32)),
        wm_g=_kn(np.ascontiguousarray(inp["Wmsa_g"].T, np.float32)),
        we_p=_kn(np.ascontiguousarray(inp["Wesm_p"].T, np.float32)),
        we_g=_kn(np.ascontiguousarray(inp["Wesm_g"].T, np.float32)),
        bfuse_p=_bc(np.concatenate([inp["bmsa_p"], np.zeros(H // 2)])),
        bfuse_g=_bc(np.concatenate([inp["bmsa_g"], np.zeros(H // 2)])),
        gbc_p=_bc(lnf_p_g), bbc_p=_bc(lnf_p_b),
        gbc_g=_bc(lnf_g_g), bbc_g=_bc(lnf_g_b),
        wl1_p=_kn(lnf_p_g[:, None] * wl1),
        wl1_g=_kn(lnf_g_g[:, None] * wl1),
        wr1_p=_kn(lnf_p_g[:, None] * wr1),
        wr1_g=_kn(lnf_g_g[:, None] * wr1),
        bl1_p=_bc(lnf_p_b @ wl1), bl1_g=_bc(lnf_g_b @ wl1),
        br1_p=_bc(lnf_p_b @ wr1), br1_g=_bc(lnf_g_b @ wr1),
        wl2=_kn(wl2), wr2=_kn(wr2),
        wcls=_kn(hg[:, None] * wcls_T),
        bcls_bc=_bc(np.asarray(inp["bcls"]) + hb @ wcls_T),
        iota_in=np.ascontiguousarray(
            np.tile(np.arange(P, dtype=np.float32)[None, :], (P, 1))),
    )

    in_maps = []
    for c in range(NCORES):
        m = dict(common)
        m["xpm"] = _pad_rows(inp["xp_msa"][c * PPC:(c + 1) * PPC], PSH)
        m["xpe"] = _pad_rows(inp["xp_esm"][c * PPC:(c + 1) * PPC], PSH)
        m["xgm"] = _pad_rows(inp["xg_msa"][c * GPC:(c + 1) * GPC], GSH)
        m["xge"] = _pad_rows(inp["xg_esm"][c * GPC:(c + 1) * GPC], GSH)
        m["gi1"], m["dl1"], m["wa1"], m["om1"] = (
            meta1[c]["g_idx"], meta1[c]["d_loc"], meta1[c]["w_a"], meta1[c]["ohme"])
        m["gi2"], m["dl2"], m["wa2"], m["om2"] = (
            meta2[c]["g_idx"], meta2[c]["d_loc"], meta2[c]["w_a"], meta2[c]["ohme"])
        in_maps.append(m)

    nc = build_program()
    res = run_bass_kernel_spmd(nc, in_maps, core_ids=list(range(NCORES)),
                               **run_kwargs)
    out = np.concatenate([res.results[c]["out"][:PPC] for c in range(NCORES)], 0)
    if run_kwargs:
        return np.ascontiguousarray(out, np.float32), res
    return np.ascontiguousarray(out, np.float32)


if __name__ == "__main__":
    import reference
    inputs = {k: np.asarray(v) for k, v in reference.setup_inputs().items()}
    got = kernel(**inputs)
    print("kernel output", got.shape, got.dtype, "finite:", np.isfinite(got).all())



# revision 9
# speedup vs baseline: 2.2378x; 2.2378x over previous
"""HGAT-ESM2 Bass kernel for 8 trn2 NeuronCores (fp16 fast path).

Key mathematical simplification: the reference's TypeAttention computes
seg_softmax(logit[d], d) where logit is per-dst-node -- values are constant
within each segment, so alpha_e = 1/indegree_r(dst_e), host-computable from
the indices alone.  The mul_W/mur_W/attn_w machinery cancels out entirely.

Remaining math per layer l (x = hom node features, table [N_hom, H]):
  hl = x @ Wl.T ; hr = x @ Wr.T
  s_e = sum_j leaky(hl[src_e, j] + hr[dst_e, j]) ; e_e = alpha_e * s_e
  a = seg_softmax(e, dst, N_hom) ; xo = segment_sum(a * x[src], dst)

Perf structure vs the f32 baseline:
  * all GEMM operands, the gathered [x || hl] tables, and the one-hot
    scatter matrices are fp16 (PE runs fp16 at 1 cycle/row vs 4 for f32;
    DMA + AllGather volume halves).  Accumulation stays f32 in PSUM, the
    logit path (t = hl+hr, Prelu sum, exp) stays f32 on DVE/ACT.
  * softmax normalization happens after the scatter (xo_un and den
    accumulate together from the same fp16 one-hot*u operand), which
    removes the third per-chunk pass of the baseline entirely.
  * fuse-phase activations arrive pre-transposed from the host, removing
    720 PE transposes + PSUM round-trips.
  * per-block chunk capacities are exact (max over cores of the actual
    edge counts) instead of worst-case constants.
"""
import sys
sys.path.insert(0, "/opt/trn_rl_repo")
import numpy as np
import concourse.bass as bass
import concourse.bacc as bacc
import concourse.mybir as mybir
import concourse.tile as tile
from concourse.bass_utils import run_bass_kernel_spmd
from concourse.masks import make_identity

F32 = mybir.dt.float32
F16 = mybir.dt.float16
I32 = mybir.dt.int32
P = 128
NCORES = 8
NpR, NgR = 40000, 20000          # real node counts
PSH, GSH = 5120, 2560            # per-core padded p/g slots
SH = PSH + GSH                   # 7680 rows per shard
NH = SH * NCORES                 # 61440 padded hom nodes
PPC, GPC = NpR // NCORES, NgR // NCORES   # 5000 / 2500 real rows per core
NB_P, NB_G = PSH // P, GSH // P  # 40 / 20 blocks per core
PAD_ROW = NH - 1                 # a guaranteed-zero row in the xh tables
H, Dm, De, CLS = 512, 256, 1280, 1024
KM, KE, KH = Dm // P, De // P, H // P
NSLAB = 5                        # row-slabs per shard for chunked AllGather
SLAB = SH // NSLAB               # 1536 rows per slab
SLABB = SLAB // P                # 12 blocks per slab
EPS = 1e-5
SLOPE = 0.2
AX = mybir.AxisListType.X
OP = mybir.AluOpType
AF = mybir.ActivationFunctionType


def _hom_ids(ids, is_g):
    ids = np.asarray(ids, np.int64)
    if is_g:
        c, r = ids // GPC, ids % GPC
        return c * SH + PSH + r
    c, r = ids // PPC, ids % PPC
    return c * SH + r


def _build_meta(rel_list, nb_g):
    """rel_list: list of (src_hom, dst_hom, w_alpha).

    Returns (per-core meta dicts, exact per-block chunk caps).
    """
    src = np.concatenate([r[0] for r in rel_list])
    dst = np.concatenate([r[1] for r in rel_list])
    w = np.concatenate([r[2] for r in rel_list]).astype(np.float32)
    core = dst // SH
    loc = dst % SH
    blk = loc // P
    dloc = (loc % P).astype(np.float32)
    nblk = NB_P + nb_g
    # exact capacity: max chunks any core needs for this block
    cnt = np.zeros((NCORES, nblk), np.int64)
    np.add.at(cnt, (core, blk), 1)
    caps = np.maximum(1, -(-cnt.max(axis=0) // P)).astype(np.int64)
    col0s = np.concatenate([[0], np.cumsum(caps)])
    ncols = int(col0s[-1])
    metas = []
    for c in range(NCORES):
        sel = np.where(core == c)[0]
        g_idx = np.full((P, ncols), PAD_ROW, np.int32)
        d_loc = np.full((P, ncols), -1.0, np.float32)
        w_a = np.zeros((P, ncols), np.float32)
        order = sel[np.argsort(blk[sel], kind="stable")]
        bvals = blk[order]
        bounds = np.searchsorted(bvals, np.arange(nblk + 1))
        for b in range(nblk):
            e = order[bounds[b]:bounds[b + 1]]
            k = len(e)
            assert k <= caps[b] * P, f"block overflow: {k} > {caps[b] * P}"
            lanes, chunks = np.arange(k) % P, np.arange(k) // P
            g_idx[lanes, int(col0s[b]) + chunks] = src[e]
            d_loc[lanes, int(col0s[b]) + chunks] = dloc[e]
            w_a[lanes, int(col0s[b]) + chunks] = w[e]
        # transposed one-hot per chunk: ohme[m, j*128 + e] = (d_loc[e, j] == m)
        ohme = (d_loc[None, :, :] == np.arange(P, dtype=np.float32)[:, None, None])
        ohme = ohme.astype(np.float16).transpose(0, 2, 1).reshape(P, ncols * P)
        metas.append(dict(g_idx=g_idx, d_loc=d_loc, w_a=w_a,
                          ohme=np.ascontiguousarray(ohme)))
    return metas, [int(x) for x in caps]


def _pad_rows(a, n):
    out = np.zeros((n, a.shape[1]), np.float32)
    out[: a.shape[0]] = np.asarray(a, np.float32)
    return out


def _kn(W, dt=np.float32):
    """[K, N] -> [128, (K//128)*N] with chunk k of rows at columns [k*N:(k+1)*N]"""
    K, N = W.shape
    return np.ascontiguousarray(
        W.reshape(K // P, P, N).transpose(1, 0, 2).reshape(P, -1).astype(dt))


def _bc(row):
    return np.ascontiguousarray(
        np.tile(np.asarray(row, np.float32)[None, :], (P, 1)))


def build_program(caps1, caps2):
    T1, T2 = sum(caps1), sum(caps2)
    C1M, C2M = max(caps1), max(caps2)
    nc = bacc.Bacc("TRN2", target_bir_lowering=False, debug=False,
                   num_devices=NCORES)

    def inp(name, shape, dt=F32):
        return nc.dram_tensor(name, list(shape), dt, kind="ExternalInput").ap()

    # pre-transposed fuse inputs (fp16), block-contiguous: block b occupies
    # cols [b*KT*P:(b+1)*KT*P], chunk k (msa 0..KM-1, esm KM..KT-1) within it
    KT = KM + KE
    xpT = inp("xpT", [P, NB_P * KT * P], F16)
    xgT = inp("xgT", [P, NB_G * KT * P], F16)
    w16names = ["wm_p", "wm_g", "we_p", "we_g",
                "wl1_p", "wl1_g", "wr1_p", "wr1_g"]
    w32names = ["bfuse_p", "bfuse_g", "gbc_p", "gbc_g", "bbc_p", "bbc_g",
                "bl1_p", "bl1_g", "br1_p", "br1_g"]
    wshapes = {"wm_p": [P, KM * (H // 2)], "wm_g": [P, KM * (H // 2)],
               "we_p": [P, KE * (H // 2)], "we_g": [P, KE * (H // 2)],
               "bfuse_p": [P, H], "bfuse_g": [P, H],
               "gbc_p": [P, H], "gbc_g": [P, H],
               "bbc_p": [P, H], "bbc_g": [P, H],
               "wl1_p": [P, KH * H], "wl1_g": [P, KH * H],
               "wr1_p": [P, KH * H], "wr1_g": [P, KH * H],
               "bl1_p": [P, H], "bl1_g": [P, H],
               "br1_p": [P, H], "br1_g": [P, H]}
    wap = {n: inp(n, wshapes[n], F16) for n in w16names}
    wap.update({n: inp(n, wshapes[n]) for n in w32names})
    wl2 = inp("wl2", [P, KH * H], F16); wr2 = inp("wr2", [P, KH * H], F16)
    wcls = inp("wcls", [P, KH * CLS], F16); bcls_bc = inp("bcls_bc", [P, CLS])
    iota_in = inp("iota_in", [P, P])
    gi1 = inp("gi1", [P, T1], I32); dl1 = inp("dl1", [P, T1]); wa1 = inp("wa1", [P, T1])
    om1 = inp("om1", [P, T1 * P], F16)
    gi2 = inp("gi2", [P, T2], I32); dl2 = inp("dl2", [P, T2]); wa2 = inp("wa2", [P, T2])
    om2 = inp("om2", [P, T2 * P], F16)
    out_d = nc.dram_tensor("out", [PSH, CLS], F32, kind="ExternalOutput").ap()

    with tile.TileContext(nc, num_cores=NCORES) as tc:
        with (
            tc.tile_pool(name="dram", bufs=1, space="DRAM") as dram,
            tc.tile_pool(name="const", bufs=1) as cpool,
        ):
            # shard tables are split into NSLAB row-slabs so each slab's
            # AllGather can launch as soon as its blocks are computed,
            # overlapping the collective with the remaining compute.
            xh1_shs = [dram.tile([SLAB, 2 * H], F16, name=f"xh1s{s}",
                                 tag=f"xh1s{s}") for s in range(NSLAB)]
            xh1_full = dram.tile([NH, 2 * H], F16, addr_space="Shared")
            xh2_shs = [dram.tile([SLAB, 2 * H], F16, name=f"xh2s{s}",
                                 tag=f"xh2s{s}") for s in range(NSLAB)]
            xh2_full = dram.tile([NH, 2 * H], F16, addr_space="Shared")
            hr1_tab = dram.tile([SH, H], F16)

            def slab_allgather(shs, full, s):
                out3 = full[:].rearrange("(c r) w -> c r w", c=NCORES)
                nc.gpsimd.collective_compute(
                    "AllGather", OP.bypass,
                    replica_groups=[list(range(NCORES))],
                    ins=[shs[s].opt()],
                    outs=[out3[:, s * SLAB:(s + 1) * SLAB, :]])

            iota = cpool.tile([P, P], F32)
            nc.sync.dma_start(out=iota[:], in_=iota_in[:])
            ident = cpool.tile([P, P], F32)
            make_identity(nc, ident[:])
            ident16 = cpool.tile([P, P], F16)
            make_identity(nc, ident16[:])
            ones16 = cpool.tile([P, 1], F16)
            nc.vector.memset(ones16[:], 1.0)

            meta_t = {}
            for name, ap, dt in (("gi1", gi1, I32), ("dl1", dl1, F32),
                                 ("wa1", wa1, F32), ("gi2", gi2, I32),
                                 ("dl2", dl2, F32), ("wa2", wa2, F32)):
                t = cpool.tile(list(ap.shape), dt, name=f"m_{name}")
                nc.sync.dma_start(out=t[:], in_=ap[:])
                meta_t[name] = t

            def transposes(src_sb, psum_pool, sbuf_pool, nchunks, tag,
                           idn, out_dt):
                """PE-transpose [128, n*128] -> list of [128,128] sbuf tiles."""
                outs = []
                for k in range(nchunks):
                    ps = psum_pool.tile([P, P], src_sb.dtype, name=f"tp_{tag}",
                                        tag="small", space="PSUM")
                    nc.tensor.transpose(out=ps[:], in_=src_sb[:, k * P:(k + 1) * P],
                                        identity=idn[:])
                    sb = sbuf_pool.tile([P, P], out_dt, name=f"ts_{tag}{k}",
                                        tag=f"ts_{tag}{k}")
                    nc.vector.tensor_copy(out=sb[:], in_=ps[:])
                    outs.append(sb)
                return outs

            def layernorm(x_sb, pool, tag):
                """returns xn tile = (x - mean)/sqrt(var+eps), per-row stats."""
                s1 = pool.tile([P, 1], F32, name=f"{tag}s1", tag=f"{tag}s1")
                nc.vector.reduce_sum(out=s1[:], in_=x_sb[:], axis=AX)
                sq = pool.tile([P, H], F32, name=f"{tag}sq", tag=f"{tag}sq")
                ssq = pool.tile([P, 1], F32, name=f"{tag}ssq", tag=f"{tag}ssq")
                nc.scalar.activation(out=sq[:], in_=x_sb[:], func=AF.Square,
                                     accum_out=ssq[:])
                mu = pool.tile([P, 1], F32, name=f"{tag}mu", tag=f"{tag}mu")
                nc.vector.tensor_scalar_mul(out=mu[:], in0=s1[:], scalar1=1.0 / H)
                var = pool.tile([P, 1], F32, name=f"{tag}var", tag=f"{tag}var")
                nc.vector.tensor_scalar_mul(out=var[:], in0=ssq[:], scalar1=1.0 / H)
                mu2 = pool.tile([P, 1], F32, name=f"{tag}mu2", tag=f"{tag}mu2")
                nc.vector.tensor_tensor(out=mu2[:], in0=mu[:], in1=mu[:], op=OP.mult)
                nc.vector.tensor_tensor(out=var[:], in0=var[:], in1=mu2[:],
                                        op=OP.subtract)
                nc.vector.tensor_scalar_add(out=var[:], in0=var[:], scalar1=EPS)
                rv = pool.tile([P, 1], F32, name=f"{tag}rv", tag=f"{tag}rv")
                nc.vector.reciprocal(out=rv[:], in_=var[:])
                rstd = pool.tile([P, 1], F32, name=f"{tag}rstd", tag=f"{tag}rstd")
                nc.scalar.activation(out=rstd[:], in_=rv[:], func=AF.Sqrt)
                xn = pool.tile([P, H], F32, name=f"{tag}xn", tag=f"{tag}xn")
                nc.vector.tensor_scalar(out=xn[:], in0=x_sb[:], scalar1=mu[:],
                                        scalar2=rstd[:], op0=OP.subtract,
                                        op1=OP.mult)
                return xn

            # ================= FUSE + layer-1 hl/hr =================
            with (
                tc.tile_pool(name="fw", bufs=1) as fw,
                tc.tile_pool(name="fsb", bufs=2) as fsb,
                tc.tile_pool(name="fps", bufs=2, space="PSUM") as fps,
            ):
                W = {}
                for n in w16names + w32names:
                    t = fw.tile(list(wshapes[n]), F16 if n in w16names else F32,
                                name=f"c_{n}")
                    nc.sync.dma_start(out=t[:], in_=wap[n][:])
                    W[n] = t
                zero_row = fw.tile([P, 2 * H], F16)
                nc.vector.memset(zero_row[:], 0.0)

                for b in range(NB_P + NB_G):
                    sfx = "p" if b < NB_P else "g"
                    bl = b if b < NB_P else b - NB_P
                    xT_src = xpT if b < NB_P else xgT
                    xT = fsb.tile([P, KT * P], F16, tag="xT", name="xT")
                    nc.sync.dma_start(
                        out=xT[:],
                        in_=xT_src[:, bl * KT * P:(bl + 1) * KT * P])
                    hps = fps.tile([P, H], F32, tag="hps", space="PSUM", name="hps")
                    for k in range(KM):
                        nc.tensor.matmul(out=hps[:, :H // 2],
                                         lhsT=xT[:, k * P:(k + 1) * P],
                                         rhs=W[f"wm_{sfx}"][:, k * (H // 2):(k + 1) * (H // 2)],
                                         start=(k == 0), stop=(k == KM - 1))
                    for k in range(KE):
                        nc.tensor.matmul(out=hps[:, H // 2:],
                                         lhsT=xT[:, (KM + k) * P:(KM + k + 1) * P],
                                         rhs=W[f"we_{sfx}"][:, k * (H // 2):(k + 1) * (H // 2)],
                                         start=(k == 0), stop=(k == KE - 1))
                    cat = fsb.tile([P, H], F32, tag="cat", name="cat")
                    nc.vector.tensor_tensor(out=cat[:], in0=hps[:],
                                            in1=W[f"bfuse_{sfx}"][:], op=OP.add)
                    xn = layernorm(cat, fsb, "f_")
                    h1 = fsb.tile([P, H], F32, tag="h1", name="h1")
                    nc.vector.tensor_tensor(out=h1[:], in0=xn[:],
                                            in1=W[f"gbc_{sfx}"][:], op=OP.mult)
                    h1h = fsb.tile([P, H], F16, tag="h1h", name="h1h")
                    nc.vector.tensor_tensor(out=h1h[:], in0=h1[:],
                                            in1=W[f"bbc_{sfx}"][:], op=OP.add)
                    nc.sync.dma_start(out=xh1_sh[b * P:(b + 1) * P, 0:H], in_=h1h[:])
                    xnT = transposes(xn, fps, fsb, KH, "xn", ident, F16)
                    for wnm, bnm, dst_ap in (
                        ("wl1", "bl1", xh1_sh[b * P:(b + 1) * P, H:2 * H]),
                        ("wr1", "br1", hr1_tab[b * P:(b + 1) * P, :]),
                    ):
                        ps = fps.tile([P, H], F32, tag="hlr", space="PSUM", name="hlr")
                        for k in range(KH):
                            nc.tensor.matmul(out=ps[:], lhsT=xnT[k][:],
                                             rhs=W[f"{wnm}_{sfx}"][:, k * H:(k + 1) * H],
                                             start=(k == 0), stop=(k == KH - 1))
                        sb = fsb.tile([P, H], F16, tag="hlr_sb", name="hlr_sb")
                        nc.vector.tensor_tensor(out=sb[:], in0=ps[:],
                                                in1=W[f"{bnm}_{sfx}"][:], op=OP.add)
                        nc.sync.dma_start(out=dst_ap, in_=sb[:])
                # zero the pad rows (so PAD_ROW gathers and pad dst reads are clean)
                for lo, hi in ((PPC, PSH), (PSH + GPC, SH)):
                    r = lo
                    while r < hi:
                        n = min(P, hi - r)
                        nc.sync.dma_start(out=xh1_sh[r:r + n, :], in_=zero_row[:n, :])
                        nc.sync.dma_start(out=xh2_sh[r:r + n, :], in_=zero_row[:n, :])
                        nc.sync.dma_start(out=hr1_tab[r:r + n, :],
                                          in_=zero_row[:n, :H])
                        r += n

            nc.gpsimd.collective_compute(
                "AllGather", OP.bypass, replica_groups=[list(range(NCORES))],
                ins=[xh1_sh.opt()], outs=[xh1_full.opt()])

            # ================= edge phase (both layers) =================
            def edge_phase(layer, xh_full, nblocks, caps, cmaxcap, gi, dl, wa,
                           om_ap, hr_fn, epilogue, xo_dt):
                with (
                    tc.tile_pool(name=f"esb{layer}", bufs=2) as esb,
                    tc.tile_pool(name=f"exs{layer}", bufs=2) as exs,
                    tc.tile_pool(name=f"eps{layer}", bufs=2, space="PSUM") as eps,
                    tc.tile_pool(name=f"exo{layer}", bufs=1, space="PSUM") as exo,
                ):
                    col0 = 0
                    for b in range(nblocks):
                        C = caps[b]
                        hr_sb = hr_fn(b, esb, eps)
                        xs = exs.tile([P, cmaxcap * 2 * H], F16, tag="xs", name="xs")
                        for j in range(C):
                            nc.gpsimd.indirect_dma_start(
                                out=xs[:, j * 2 * H:(j + 1) * 2 * H],
                                out_offset=None, in_=xh_full[:],
                                in_offset=bass.IndirectOffsetOnAxis(
                                    ap=meta_t[gi][:, col0 + j:col0 + j + 1], axis=0))
                        om_blk = esb.tile([P, cmaxcap * P], F16, tag="om", name="om_blk")
                        nc.sync.dma_start(out=om_blk[:, :C * P],
                                          in_=om_ap[:, col0 * P:(col0 + C) * P])
                        e_all = esb.tile([P, cmaxcap], F32, tag="e_all", name="e_all")
                        oh_all = esb.tile([P, cmaxcap * P], F16, tag="oh_all",
                                          name="oh_all")
                        kmax = esb.tile([P, 1], F32, tag="kmax", name="kmax")
                        nc.vector.memset(kmax[:], 0.0)
                        for j in range(C):
                            he = eps.tile([P, H], F32, tag="big", space="PSUM", name="he")
                            nc.tensor.matmul(out=he[:], lhsT=om_blk[:, j * P:(j + 1) * P],
                                             rhs=hr_sb[:], start=True, stop=True)
                            t = esb.tile([P, H], F32, tag="t", name="t")
                            nc.vector.tensor_tensor(
                                out=t[:], in0=xs[:, j * 2 * H + H:(j + 1) * 2 * H],
                                in1=he[:], op=OP.add)
                            lk = esb.tile([P, H], F32, tag="lk", name="lk")
                            s_col = esb.tile([P, 1], F32, tag="s_col", name="s_col")
                            nc.scalar.activation(out=lk[:], in_=t[:], func=AF.Prelu,
                                                 alpha=SLOPE, accum_out=s_col[:])
                            nc.vector.tensor_tensor(
                                out=e_all[:, j:j + 1], in0=s_col[:],
                                in1=meta_t[wa][:, col0 + j:col0 + j + 1], op=OP.mult)
                            nc.vector.tensor_tensor(
                                out=oh_all[:, j * P:(j + 1) * P],
                                in0=meta_t[dl][:, col0 + j:col0 + j + 1].to_broadcast([P, P]),
                                in1=iota[:], op=OP.is_equal)
                            m2 = esb.tile([P, P], F16, tag="m2", name="m2")
                            nc.vector.tensor_scalar(
                                out=m2[:], in0=oh_all[:, j * P:(j + 1) * P],
                                scalar1=e_all[:, j:j + 1], scalar2=None, op0=OP.mult)
                            m2T = eps.tile([P, P], F16, tag="small", space="PSUM",
                                           name="m2T")
                            nc.tensor.transpose(out=m2T[:], in_=m2[:],
                                                identity=ident16[:])
                            cmax = esb.tile([P, 1], F32, tag="cmax", name="cmax")
                            nc.vector.tensor_reduce(out=cmax[:], in_=m2T[:], axis=AX,
                                                    op=OP.max)
                            nc.vector.tensor_tensor(out=kmax[:], in0=kmax[:],
                                                    in1=cmax[:], op=OP.max)
                        negk = esb.tile([P, 1], F32, tag="negk", name="negk")
                        nc.vector.tensor_scalar_mul(out=negk[:], in0=kmax[:],
                                                    scalar1=-1.0)
                        negk16 = esb.tile([P, 1], F16, tag="negk16", name="negk16")
                        nc.vector.tensor_copy(out=negk16[:], in_=negk[:])
                        den = exo.tile([P, 1], F32, tag="den", space="PSUM", name="den")
                        xo_ps = exo.tile([P, H], F32, tag="xo", space="PSUM", name="xo")
                        for j in range(C):
                            nke = eps.tile([P, 1], F32, tag="small", space="PSUM",
                                           name="nke")
                            nc.tensor.matmul(out=nke[:], lhsT=om_blk[:, j * P:(j + 1) * P],
                                             rhs=negk16[:], start=True, stop=True)
                            nke_sb = esb.tile([P, 1], F32, tag="nke_sb", name="nke_sb")
                            nc.vector.tensor_copy(out=nke_sb[:], in_=nke[:])
                            u_col = esb.tile([P, 1], F32, tag="u_col", name="u_col")
                            nc.scalar.activation(out=u_col[:],
                                                 in_=e_all[:, j:j + 1], func=AF.Exp,
                                                 bias=nke_sb[:])
                            ohu = esb.tile([P, P], F16, tag="ohu", name="ohu")
                            nc.vector.tensor_scalar(
                                out=ohu[:], in0=oh_all[:, j * P:(j + 1) * P],
                                scalar1=u_col[:], scalar2=None, op0=OP.mult)
                            nc.tensor.matmul(out=xo_ps[:], lhsT=ohu[:],
                                             rhs=xs[:, j * 2 * H:j * 2 * H + H],
                                             start=(j == 0), stop=(j == C - 1))
                            nc.tensor.matmul(out=den[:], lhsT=ohu[:],
                                             rhs=ones16[:],
                                             start=(j == 0), stop=(j == C - 1))
                        den_sb = esb.tile([P, 1], F32, tag="den_sb", name="den_sb")
                        nc.vector.tensor_scalar_max(out=den_sb[:], in0=den[:],
                                                    scalar1=1e-30)
                        rden = esb.tile([P, 1], F32, tag="rden", name="rden")
                        nc.vector.reciprocal(out=rden[:], in_=den_sb[:])
                        xo_sb = esb.tile([P, H], xo_dt, tag="xo_sb", name="xo_sb")
                        nc.vector.tensor_scalar(out=xo_sb[:], in0=xo_ps[:],
                                                scalar1=rden[:], scalar2=None,
                                                op0=OP.mult)
                        epilogue(b, xo_sb, esb, eps)
                        col0 += C

            with tc.tile_pool(name="l2w", bufs=1) as l2w:
                wl2_t = l2w.tile([P, KH * H], F16, name="wl2_t")
                nc.sync.dma_start(out=wl2_t[:], in_=wl2[:])
                wr2_t = l2w.tile([P, KH * H], F16, name="wr2_t")
                nc.sync.dma_start(out=wr2_t[:], in_=wr2[:])

                # ---- layer 1 ----
                def hr1_fn(b, esb, eps):
                    hr_sb = esb.tile([P, H], F16, tag="hr_sb", name="hr_sb")
                    nc.sync.dma_start(out=hr_sb[:], in_=hr1_tab[b * P:(b + 1) * P, :])
                    return hr_sb

                def epi1(b, xo_sb, esb, eps):
                    nc.sync.dma_start(out=xh2_sh[b * P:(b + 1) * P, 0:H], in_=xo_sb[:])
                    xoT = transposes(xo_sb, eps, esb, KH, "xoT", ident16, F16)
                    ps = eps.tile([P, H], F32, tag="big", space="PSUM", name="hl2ps")
                    for k in range(KH):
                        nc.tensor.matmul(out=ps[:], lhsT=xoT[k][:],
                                         rhs=wl2_t[:, k * H:(k + 1) * H],
                                         start=(k == 0), stop=(k == KH - 1))
                    sb = esb.tile([P, H], F16, tag="hl2sb", name="hl2sb")
                    nc.vector.tensor_copy(out=sb[:], in_=ps[:])
                    nc.sync.dma_start(out=xh2_sh[b * P:(b + 1) * P, H:2 * H], in_=sb[:])

                edge_phase(1, xh1_full, NB_P + NB_G, caps1, max(caps1),
                           "gi1", "dl1", "wa1", om1, hr1_fn, epi1, F16)

                nc.gpsimd.collective_compute(
                    "AllGather", OP.bypass, replica_groups=[list(range(NCORES))],
                    ins=[xh2_sh.opt()], outs=[xh2_full.opt()])

                # ---- layer 2 + head LN + cls ----
                with tc.tile_pool(name="clsw", bufs=1) as clsw:
                    wcls_t = clsw.tile([P, KH * CLS], F16, name="wcls_t")
                    nc.sync.dma_start(out=wcls_t[:], in_=wcls[:])
                    bcls_t = clsw.tile([P, CLS], F32, name="bcls_t")
                    nc.sync.dma_start(out=bcls_t[:], in_=bcls_bc[:])

                    def hr2_fn(b, esb, eps):
                        xr = esb.tile([P, H], F16, tag="xr", name="xr")
                        nc.sync.dma_start(out=xr[:],
                                          in_=xh2_sh[b * P:(b + 1) * P, 0:H])
                        xrT = transposes(xr, eps, esb, KH, "xrT", ident16, F16)
                        ps = eps.tile([P, H], F32, tag="big", space="PSUM",
                                      name="hr2ps")
                        for k in range(KH):
                            nc.tensor.matmul(out=ps[:], lhsT=xrT[k][:],
                                             rhs=wr2_t[:, k * H:(k + 1) * H],
                                             start=(k == 0), stop=(k == KH - 1))
                        hr_sb = esb.tile([P, H], F16, tag="hr_sb", name="hr_sb")
                        nc.vector.tensor_copy(out=hr_sb[:], in_=ps[:])
                        return hr_sb

                    def epi2(b, xo_sb, esb, eps):
                        xn = layernorm(xo_sb, esb, "c_")
                        xnT = transposes(xn, eps, esb, KH, "c_xnT", ident, F16)
                        for half in range(2):
                            ps = eps.tile([P, H], F32, tag="big", space="PSUM",
                                          name="clsps")
                            for k in range(KH):
                                nc.tensor.matmul(
                                    out=ps[:], lhsT=xnT[k][:],
                                    rhs=wcls_t[:, k * CLS + half * H:
                                               k * CLS + (half + 1) * H],
                                    start=(k == 0), stop=(k == KH - 1))
                            ob = esb.tile([P, H], F32, tag="c_ob", name="c_ob")
                            nc.vector.tensor_tensor(
                                out=ob[:], in0=ps[:],
                                in1=bcls_t[:, half * H:(half + 1) * H], op=OP.add)
                            nc.sync.dma_start(
                                out=out_d[b * P:(b + 1) * P,
                                          half * H:(half + 1) * H],
                                in_=ob[:])

                    edge_phase(2, xh2_full, NB_P, caps2, max(caps2),
                               "gi2", "dl2", "wa2", om2, hr2_fn, epi2, F32)

    nc.compile()
    return nc


def kernel(_run_kwargs=None, **inputs):
    run_kwargs = _run_kwargs or {}
    inp = {k: np.asarray(v) for k, v in inputs.items()}

    def rel_arrays(st, dt, s, d):
        nt = NpR if dt == 0 else NgR
        ideg = np.bincount(d, minlength=nt)
        w = (1.0 / np.maximum(ideg[d], 1)).astype(np.float32)
        return (_hom_ids(s, st == 1), _hom_ids(d, dt == 1), w)

    rels = [(0, 0, inp["pp_src"], inp["pp_dst"]),
            (0, 1, inp["pg_src"], inp["pg_dst"]),
            (1, 0, inp["gp_src"], inp["gp_dst"]),
            (1, 1, inp["gg_src"], inp["gg_dst"])]
    rel1 = [rel_arrays(*r) for r in rels]
    meta1, caps1 = _build_meta(rel1, NB_G)
    rel2 = [rel_arrays(*r) for r in rels if r[1] == 0]
    meta2, caps2 = _build_meta(rel2, 0)

    lnf_p_g, lnf_p_b = inp["lnf_p_g"], inp["lnf_p_b"]
    lnf_g_g, lnf_g_b = inp["lnf_g_g"], inp["lnf_g_b"]
    wl1 = np.ascontiguousarray(inp["node_Wl"][0].T, dtype=np.float32)
    wr1 = np.ascontiguousarray(inp["node_Wr"][0].T, dtype=np.float32)
    wl2 = np.ascontiguousarray(inp["node_Wl"][1].T, dtype=np.float32)
    wr2 = np.ascontiguousarray(inp["node_Wr"][1].T, dtype=np.float32)
    hg, hb = inp["head_ln_g"], inp["head_ln_b"]
    wcls_T = np.ascontiguousarray(inp["Wcls"].T, dtype=np.float32)

    common = dict(
        wm_p=_kn(np.ascontiguousarray(inp["Wmsa_p"].T), np.float16),
        wm_g=_kn(np.ascontiguousarray(inp["Wmsa_g"].T), np.float16),
        we_p=_kn(np.ascontiguousarray(inp["Wesm_p"].T), np.float16),
        we_g=_kn(np.ascontiguousarray(inp["Wesm_g"].T), np.float16),
        bfuse_p=_bc(np.concatenate([inp["bmsa_p"], np.zeros(H // 2)])),
        bfuse_g=_bc(np.concatenate([inp["bmsa_g"], np.zeros(H // 2)])),
        gbc_p=_bc(lnf_p_g), bbc_p=_bc(lnf_p_b),
        gbc_g=_bc(lnf_g_g), bbc_g=_bc(lnf_g_b),
        wl1_p=_kn(lnf_p_g[:, None] * wl1, np.float16),
        wl1_g=_kn(lnf_g_g[:, None] * wl1, np.float16),
        wr1_p=_kn(lnf_p_g[:, None] * wr1, np.float16),
        wr1_g=_kn(lnf_g_g[:, None] * wr1, np.float16),
        bl1_p=_bc(lnf_p_b @ wl1), bl1_g=_bc(lnf_g_b @ wl1),
        br1_p=_bc(lnf_p_b @ wr1), br1_g=_bc(lnf_g_b @ wr1),
        wl2=_kn(wl2, np.float16), wr2=_kn(wr2, np.float16),
        wcls=_kn(hg[:, None] * wcls_T, np.float16),
        bcls_bc=_bc(np.asarray(inp["bcls"]) + hb @ wcls_T),
        iota_in=np.ascontiguousarray(
            np.tile(np.arange(P, dtype=np.float32)[None, :], (P, 1))),
    )

    def _xT_blocks(xm, xe):
        """[rows, Dm]+[rows, De] -> [128, nb*(KM+KE)*128] block-contiguous
        transposed layout: out[d, (b*(KM+KE)+k)*128 + r] = x[b*128+r, k*128+d]."""
        nb = xm.shape[0] // P
        am = np.asarray(xm, np.float16).reshape(nb, P, KM, P).transpose(3, 0, 2, 1)
        ae = np.asarray(xe, np.float16).reshape(nb, P, KE, P).transpose(3, 0, 2, 1)
        return np.ascontiguousarray(
            np.concatenate([am, ae], axis=2).reshape(P, -1))

    in_maps = []
    for c in range(NCORES):
        m = dict(common)
        m["xpT"] = _xT_blocks(
            _pad_rows(inp["xp_msa"][c * PPC:(c + 1) * PPC], PSH),
            _pad_rows(inp["xp_esm"][c * PPC:(c + 1) * PPC], PSH))
        m["xgT"] = _xT_blocks(
            _pad_rows(inp["xg_msa"][c * GPC:(c + 1) * GPC], GSH),
            _pad_rows(inp["xg_esm"][c * GPC:(c + 1) * GPC], GSH))
        m["gi1"], m["dl1"], m["wa1"], m["om1"] = (
            meta1[c]["g_idx"], meta1[c]["d_loc"], meta1[c]["w_a"], meta1[c]["ohme"])
        m["gi2"], m["dl2"], m["wa2"], m["om2"] = (
            meta2[c]["g_idx"], meta2[c]["d_loc"], meta2[c]["w_a"], meta2[c]["ohme"])
        in_maps.append(m)

    nc = build_program(caps1, caps2)
    res = run_bass_kernel_spmd(nc, in_maps, core_ids=list(range(NCORES)),
                               **run_kwargs)
    out = np.concatenate([res.results[c]["out"][:PPC] for c in range(NCORES)], 0)
    if run_kwargs:
        return np.ascontiguousarray(out, np.float32), res
    return np.ascontiguousarray(out, np.float32)


if __name__ == "__main__":
    import reference
    inputs = {k: np.asarray(v) for k, v in reference.setup_inputs().items()}
    got = kernel(**inputs)
    print("kernel output", got.shape, got.dtype, "finite:", np.isfinite(got).all())
